# revision 8
# baseline (speedup 1.0000x reference)
"""Trainium2 Bass kernel for nn_Decoder (Tacotron-style attention decoder).

8-way model parallelism on one trn2 chip:
- LSTM gate dims split across cores (att: 128 hidden/core, dec: 64/core),
  proj 20 rows/core. Weights SBUF-resident fp32 as the MOVING matmul
  operand (weights stream at 1 col/cycle regardless of dtype, so fp32 is
  free vs bf16 in matmul time).
- Full batch B=64 on every core as the stationary operand.
- alpha recursion + context einsum batch-split (8 rows/core, batch-major
  alpha: the shift is a free-dim offset AP; per-batch scalars are
  per-partition tensor_scalar operands).
- One AllGather per step carries q(t) | dh(t-1) | ctx(t) through DRAM;
  PE transposes rebuild dim-major stationaries.
- ta (u, sq) folded into the dec matmul as 2 extra output columns; each
  core extracts its own batch slice with a per-core selector matmul.
- prenet on host; sharded upload + one AllGather at start.
"""
import os
import sys

sys.path.insert(0, "/opt/trn_rl_repo")
import numpy as np

B, T_ENC, CTX_DIM = 64, 512, 512
T_FRAMES, MEM_DIM, R = 1000, 80, 2
QUERY_DIM, DEC_DIM, PRENET_DIM = 1024, 512, 256
MEMR = MEM_DIM * R
BN = 1.0 / np.sqrt(1.0 + 1e-5)
S_FULL = T_FRAMES // R  # 500
NCORES = 8
BL = B // NCORES         # 8 batch rows per core (alpha/einsum split)
HQ = QUERY_DIM // NCORES  # 128 att hidden per core
HD = DEC_DIM // NCORES    # 64 dec hidden per core
PR = MEMR // NCORES       # 20 proj rows per core
S_PAD = 504               # padded step count for prenet allgather (504/8=63)

ATT_KT = 15   # 0-1 pre | 2 bias | 3-10 q | 11-14 ctx
DEC_KT = 17   # 0-7 q | 8-11 ctx | 12-15 dh | 16 bias
PROJ_KT = 9   # 0-3 dh | 4-7 ctx | 8 bias

QOFF = 0                      # fp32 elem offsets in the AG piece
DHOFF = 128 * B               # 8192
CTXOFF = DHOFF + HD * B       # 12288
PIECE = CTXOFF + BL * T_ENC   # 16384


def _gate_rows(G, c, gpc):
    rows = []
    for gate in range(4):
        rows.extend(range(gate * G + c * gpc, gate * G + (c + 1) * gpc))
    return np.array(rows)


def prep_inputs(inputs):
    """Host-side prep (numpy): returns per-core in_maps."""
    f = lambda k: np.asarray(inputs[k], np.float32)
    att_wih, att_whh = f("att_wih"), f("att_whh")
    att_b = f("att_bih") + f("att_bhh")
    dec_wih, dec_whh = f("dec_wih"), f("dec_whh")
    dec_b = f("dec_bih") + f("dec_bhh")
    proj_w, proj_b = f("proj_w"), f("proj_b")
    ta_u_w, ta_u_b = f("ta_u_w"), f("ta_u_b")
    ta_sq_w, ta_sq_b = f("ta_sq_w"), f("ta_sq_b")
    inp = f("inputs")

    mem_steps = f("memory").reshape(B, S_FULL, MEMR).transpose(1, 0, 2)
    mem_steps = np.concatenate(
        [np.broadcast_to(f("mem_init"), (1, B, MEMR)), mem_steps], 0)
    h1 = np.maximum(mem_steps @ f("prenet_w1").T, 0.0) * BN
    pre = (np.maximum(h1 @ f("prenet_w2").T, 0.0) * BN)[:-1]  # [500, 64, 256]
    preT = np.zeros((S_PAD, 2, 128, B), np.float32)
    preT[:S_FULL] = pre.transpose(0, 2, 1).reshape(S_FULL, 2, 128, B)

    in_maps = []
    for c in range(NCORES):
        m = {}
        rows = _gate_rows(QUERY_DIM, c, HQ)
        Wc = np.zeros((4 * HQ, ATT_KT * 128), np.float32)
        Wc[:, 0:256] = att_wih[rows, 0:256]
        Wc[:, 256] = att_b[rows]
        Wc[:, 384:1408] = att_whh[rows]
        Wc[:, 1408:1920] = att_wih[rows, 256:768]
        m["w_att"] = np.ascontiguousarray(Wc.T.reshape(ATT_KT, 128, 4 * HQ))

        rows = _gate_rows(DEC_DIM, c, HD)
        Wd = np.zeros((4 * HD + 2, DEC_KT * 128), np.float32)
        Wd[:4 * HD, 0:1024] = dec_wih[rows, 0:1024]
        Wd[:4 * HD, 1024:1536] = dec_wih[rows, 1024:1536]
        Wd[:4 * HD, 1536:2048] = dec_whh[rows]
        Wd[:4 * HD, 2048] = dec_b[rows]
        Wd[4 * HD + 0, 0:1024] = ta_u_w[0, 512:1536]
        Wd[4 * HD + 0, 1024:1536] = ta_u_w[0, 0:512]
        Wd[4 * HD + 0, 2048] = ta_u_b[0]
        Wd[4 * HD + 1, 0:1024] = ta_sq_w[0, 512:1536]
        Wd[4 * HD + 1, 1024:1536] = ta_sq_w[0, 0:512]
        Wd[4 * HD + 1, 2048] = ta_sq_b[0]
        m["w_dec"] = np.ascontiguousarray(Wd.T.reshape(DEC_KT, 128, 4 * HD + 2))

        prows = np.arange(c * PR, (c + 1) * PR)
        Wp = np.zeros((PR, PROJ_KT * 128), np.float32)
        Wp[:, 0:512] = proj_w[prows, 0:512]
        Wp[:, 512:1024] = proj_w[prows, 512:1024]
        Wp[:, 1024] = proj_b[prows]
        m["w_proj"] = np.ascontiguousarray(Wp.T.reshape(PROJ_KT, 128, PR))

        m["inp"] = np.ascontiguousarray(
            inp[c * BL:(c + 1) * BL].reshape(BL, 4, 128, CTX_DIM))
        m["pre"] = np.ascontiguousarray(
            preT[c * (S_PAD // NCORES):(c + 1) * (S_PAD // NCORES)])

        m["qT0"] = np.ascontiguousarray(np.broadcast_to(
            f("q_init").reshape(8, 128).T[:, :, None], (128, 8, B)).astype(np.float32))
        m["ctxT0"] = np.ascontiguousarray(np.broadcast_to(
            f("ctx_init").reshape(4, 128).T[:, :, None], (128, 4, B)).astype(np.float32))
        m["dh_piece0"] = np.ascontiguousarray(np.broadcast_to(
            f("dh_init")[c * HD:(c + 1) * HD][:, None], (HD, B)).astype(np.float32))

        a0 = np.full((BL, T_ENC), 1e-7, np.float32)
        a0[:, 0] = 1.0
        m["alpha0"] = a0
        m["uinit"] = np.array([[0.5, 1.4]] * BL, np.float32)  # [BL,2]: u0, sq0
        ones_col = np.zeros((128, B), np.float32)
        ones_col[0, :] = 1.0
        m["ones_col"] = ones_col
        m["ident"] = np.eye(128, dtype=np.float32)
        sel = np.zeros((B, BL), np.float32)
        sel[c * BL:(c + 1) * BL, :] = np.eye(BL)
        m["sel"] = sel
        in_maps.append(m)
    return in_maps


def build(S, use_tile_position=True):
    import concourse.bacc as bacc
    import concourse.mybir as mybir
    import concourse.tile as tile

    AF = mybir.ActivationFunctionType
    ALU = mybir.AluOpType
    dt = mybir.dt

    nc = bacc.Bacc("TRN2", target_bir_lowering=False, debug=False,
                   num_devices=NCORES)

    def din(name, shape):
        return nc.dram_tensor(name, list(shape), dt.float32,
                              kind="ExternalInput").ap()

    w_att_d = din("w_att", (ATT_KT, 128, 4 * HQ))
    w_dec_d = din("w_dec", (DEC_KT, 128, 4 * HD + 2))
    w_proj_d = din("w_proj", (PROJ_KT, 128, PR))
    inp_d = din("inp", (BL, 4, 128, CTX_DIM))
    pre_d = din("pre", (S_PAD // NCORES, 2, 128, B))
    qT0_d = din("qT0", (128, 8, B))
    ctxT0_d = din("ctxT0", (128, 4, B))
    dhp0_d = din("dh_piece0", (HD, B))
    alpha0_d = din("alpha0", (BL, T_ENC))
    uinit_d = din("uinit", (BL, 2))
    ones_d = din("ones_col", (128, B))
    ident_d = din("ident", (128, 128))
    sel_d = din("sel", (B, BL))

    outs_d = nc.dram_tensor("outs", [S, B, PR], dt.float32,
                            kind="ExternalOutput").ap()
    aligns_d = nc.dram_tensor("aligns", [S, BL, T_ENC], dt.float32,
                              kind="ExternalOutput").ap()

    RG = [list(range(NCORES))]

    with tile.TileContext(nc) as tc:
        with (
            tc.tile_pool(name="const", bufs=1) as cpool,
            tc.tile_pool(name="state", bufs=1) as spool,
            tc.tile_pool(name="loop", bufs=3) as lpool,
            tc.tile_pool(name="nl", bufs=2) as nlpool,
            tc.tile_pool(name="ps_att", bufs=1, space="PSUM") as ps_att,
            tc.tile_pool(name="ps_dec", bufs=2, space="PSUM") as ps_dec,
            tc.tile_pool(name="ps_ein", bufs=1, space="PSUM") as ps_ein,
            tc.tile_pool(name="ps_sm", bufs=3, space="PSUM") as ps_sm,
            tc.tile_pool(name="dram", bufs=3, space="DRAM") as dpool,
        ):
            # ---------- constants / weights ----------
            wa = cpool.tile([128, ATT_KT, 4 * HQ], dt.float32, name="wa")
            nc.sync.dma_start(wa[:], w_att_d.rearrange("k p g -> p k g"))
            wd = cpool.tile([128, DEC_KT, 4 * HD + 2], dt.float32, name="wd")
            nc.sync.dma_start(wd[:], w_dec_d.rearrange("k p g -> p k g"))
            wp = cpool.tile([128, PROJ_KT, PR], dt.float32, name="wp")
            nc.sync.dma_start(wp[:], w_proj_d.rearrange("k p g -> p k g"))
            inp_sb = cpool.tile([128, BL, 4, CTX_DIM], dt.float32, name="inp_sb")
            nc.sync.dma_start(inp_sb[:], inp_d.rearrange("b k p d -> p b k d"))
            ones_sb = cpool.tile([128, B], dt.float32, name="ones_sb")
            nc.sync.dma_start(ones_sb[:], ones_d)
            ident = cpool.tile([128, 128], dt.float32, name="ident")
            nc.sync.dma_start(ident[:], ident_d)
            sel_sb = cpool.tile([B, BL], dt.float32, name="sel_sb")
            nc.sync.dma_start(sel_sb[:], sel_d)

            # prenet allgather
            pre_in = dpool.tile([S_PAD // NCORES, 2, 128, B], dt.float32,
                                name="pre_in")
            nc.sync.dma_start(pre_in[:], pre_d)
            pre_full = dpool.tile([S_PAD, 2, 128, B], dt.float32, name="pre_full")
            nc.gpsimd.collective_compute(
                "AllGather", ALU.bypass, replica_groups=RG,
                ins=[pre_in[:].opt()], outs=[pre_full[:].opt()])

            # ---------- persistent state ----------
            c_att = spool.tile([B, HQ], dt.float32, name="c_att")
            nc.vector.memset(c_att[:], 0.0)
            c_dec = spool.tile([B, HD], dt.float32, name="c_dec")
            nc.vector.memset(c_dec[:], 0.0)
            qT = spool.tile([128, 8, B], dt.float32, name="qT_init")
            nc.sync.dma_start(qT[:], qT0_d)
            ctxT = spool.tile([128, 4, B], dt.float32, name="ctxT_init")
            nc.sync.dma_start(ctxT[:], ctxT0_d)
            dhT = None  # assembled from AG each step
            alpha = spool.tile([BL, T_ENC], dt.float32, name="alpha_init")
            nc.sync.dma_start(alpha[:], alpha0_d)
            usq0 = spool.tile([BL, 2], dt.float32, name="usq_init")
            nc.sync.dma_start(usq0[:], uinit_d)
            eps_col = spool.tile([BL, 1], dt.float32, name="eps_col")
            nc.vector.memset(eps_col[:], 1e-6)

            alpha_bufs = [spool.tile([BL, T_ENC], dt.float32, name="alpha_a"),
                          spool.tile([BL, T_ENC], dt.float32, name="alpha_b")]

            ps_dec_prev = None
            ctxT_prev = None
            dh_piece0 = spool.tile([HD, B], dt.float32, name="dh_piece0")
            nc.sync.dma_start(dh_piece0[:], dhp0_d)

            def lstm_nonlin(ps, G, cstate, tag):
                """batch-major LSTM cell tail: returns h [B, G]."""
                sig_if = nlpool.tile([B, 2 * G], dt.float32, name=f"{tag}_sif")
                nc.scalar.activation(sig_if[:], ps[:, 0:2 * G], AF.Sigmoid)
                tng = nlpool.tile([B, G], dt.float32, name=f"{tag}_tng")
                nc.scalar.activation(tng[:], ps[:, 2 * G:3 * G], AF.Tanh)
                t1 = nlpool.tile([B, G], dt.float32, name=f"{tag}_t1")
                nc.vector.tensor_tensor(t1[:], sig_if[:, G:2 * G], cstate[:], ALU.mult)
                t2 = nlpool.tile([B, G], dt.float32, name=f"{tag}_t2")
                nc.vector.tensor_tensor(t2[:], sig_if[:, 0:G], tng[:], ALU.mult)
                nc.vector.tensor_tensor(cstate[:], t1[:], t2[:], ALU.add)
                th = nlpool.tile([B, G], dt.float32, name=f"{tag}_th")
                nc.scalar.activation(th[:], cstate[:], AF.Tanh)
                return th

            for t in range(S):
                # ============ att matmul (t) ============
                preT_t = lpool.tile([128, 2, B], dt.float32, name="preT_t")
                nc.sync.dma_start(preT_t[:],
                                  pre_full[t].rearrange("k p b -> p k b"))
                g_att = ps_att.tile([B, 4 * HQ], dt.float32, name="g_att")
                att_pieces = ([preT_t[:, 0, :], preT_t[:, 1, :], ones_sb[:]]
                              + [qT[:, j, :] for j in range(8)]
                              + [ctxT[:, j, :] for j in range(4)])
                for kt in range(ATT_KT):
                    nc.tensor.matmul(g_att[:], att_pieces[kt], wa[:, kt, :],
                                     start=(kt == 0), stop=(kt == ATT_KT - 1))

                # ============ SIG phase ============
                th_a = lstm_nonlin(g_att, HQ, c_att, "a")
                sig_o = nlpool.tile([B, HQ], dt.float32, name="a_so")
                nc.scalar.activation(sig_o[:], g_att[:, 3 * HQ:4 * HQ], AF.Sigmoid)
                q_bm = nlpool.tile([B, HQ], dt.float32, name="q_bm")
                nc.vector.tensor_tensor(q_bm[:], sig_o[:], th_a[:], ALU.mult)
                ps_qT = ps_sm.tile([128, B], dt.float32, name="ps_qT", tag="sm")
                nc.tensor.transpose(ps_qT[:], q_bm[:], ident[0:B, 0:B])
                q_piece = lpool.tile([128, B], dt.float32, name="q_piece")
                nc.scalar.copy(q_piece[:], ps_qT[:])

                if ps_dec_prev is not None:
                    th_d = lstm_nonlin(ps_dec_prev, HD, c_dec, "d")
                    dso = nlpool.tile([B, HD + 2], dt.float32, name="d_so")
                    nc.scalar.activation(dso[:], ps_dec_prev[:, 3 * HD:4 * HD + 2],
                                         AF.Sigmoid)
                    dh_bm = nlpool.tile([B, HD], dt.float32, name="dh_bm")
                    nc.vector.tensor_tensor(dh_bm[:], dso[:, 0:HD], th_d[:], ALU.mult)
                    ps_dhT = ps_sm.tile([128, B], dt.float32, name="ps_dhT", tag="sm")
                    nc.tensor.transpose(ps_dhT[0:HD, :], dh_bm[:], ident[0:B, 0:B])
                    dh_piece = lpool.tile([HD, B], dt.float32, name="dh_piece")
                    nc.scalar.copy(dh_piece[:], ps_dhT[0:HD, :])
                    # select this core's u, sq_raw -> [BL, 2]
                    ps_usq = ps_sm.tile([BL, 2], dt.float32, name="ps_usq", tag="sm")
                    nc.tensor.matmul(ps_usq[:], sel_sb[:], dso[:, HD:HD + 2],
                                     start=True, stop=True)
                    usq = nlpool.tile([BL, 2], dt.float32, name="usq")
                    nc.vector.tensor_copy(usq[:], ps_usq[:])
                    u_col = usq[:, 0:1]
                    sq_is_raw = True
                else:
                    dh_piece = dh_piece0
                    usq = usq0
                    u_col = usq[:, 0:1]
                    sq_is_raw = False

                # ============ alpha (t) ============
                um1 = nlpool.tile([BL, 1], dt.float32, name="um1")
                nc.vector.tensor_scalar(um1[:], u_col, -1.0, 1.0, ALU.mult, ALU.add)
                sq1 = nlpool.tile([BL, 1], dt.float32, name="sq1")
                if sq_is_raw:
                    nc.vector.tensor_scalar(sq1[:], usq[:, 1:2], 1.0, None, ALU.add)
                else:
                    nc.vector.tensor_copy(sq1[:], usq[:, 1:2])
                t1a = nlpool.tile([BL, T_ENC], dt.float32, name="alpha_t1")
                nc.vector.tensor_scalar(t1a[:], alpha[:], um1[:].opt(), None, ALU.mult)
                mix = nlpool.tile([BL, T_ENC], dt.float32, name="alpha_mix")
                nc.vector.scalar_tensor_tensor(
                    mix[:, 1:T_ENC], alpha[:, 0:T_ENC - 1], u_col.opt(),
                    t1a[:, 1:T_ENC], ALU.mult, ALU.add)
                nc.vector.tensor_copy(mix[:, 0:1], t1a[:, 0:1])
                ln_a = nlpool.tile([BL, T_ENC], dt.float32, name="ln_a")
                nc.scalar.activation(ln_a[:], mix[:], AF.Ln, bias=eps_col[:])
                alpha_new = alpha_bufs[t % 2]
                Z = nlpool.tile([BL, 1], dt.float32, name="Z")
                A_pow = nlpool.tile([BL, T_ENC], dt.float32, name="A_pow")
                nc.scalar.activation(A_pow[:], ln_a[:], AF.Exp,
                                     scale=sq1[:].opt(), accum_out=Z[:])
                rcpZ = nlpool.tile([BL, 1], dt.float32, name="rcpZ")
                nc.vector.reciprocal(rcpZ[:], Z[:])
                nc.vector.tensor_scalar(alpha_new[:], A_pow[:], rcpZ[:].opt(),
                                        None, ALU.mult)
                nc.sync.dma_start(aligns_d[t], alpha_new[:])
                alpha = alpha_new

                # alpha transpose -> [128, 4, 40] (cols 8-39 garbage pad)
                ps_aT = ps_sm.tile([128, 4 * BL], dt.float32, name="ps_aT", tag="sm")
                for j in range(4):
                    nc.tensor.transpose(ps_aT[:, j * BL:(j + 1) * BL],
                                        alpha[:, j * 128:(j + 1) * 128],
                                        ident[0:BL, 0:BL])
                aT = lpool.tile([128, 4, 40], dt.float32, name="aT")
                nc.vector.memset(aT[:], 0.0)
                for j in range(4):
                    nc.vector.tensor_copy(aT[:, j, 0:BL],
                                          ps_aT[:, j * BL:(j + 1) * BL])

                # ============ einsum (t) ============
                ps_ctx = ps_ein.tile([128, 1024], dt.float32, name="ps_ctx")
                for b in range(BL):
                    r, g = b // 4, b % 4
                    st = 3 * r + g
                    for kt in range(4):
                        nc.tensor.matmul(
                            ps_ctx[32 * g:32 * (g + 1), 512 * r:512 * (r + 1)],
                            aT[:, kt, st:st + 32], inp_sb[:, b, kt, :],
                            start=(kt == 0), stop=(kt == 3),
                            tile_position=(0, 32 * g) if use_tile_position else None)
                ctx_scat = lpool.tile([128, 1024], dt.float32, name="ctx_scat")
                nc.vector.tensor_copy(ctx_scat[:, 0:512], ps_ctx[:, 0:512])
                nc.scalar.copy(ctx_scat[:, 512:1024], ps_ctx[:, 512:1024])

                # ============ AG (t): q(t) | dh(t-1) | ctx(t) ============
                bounce_in = dpool.tile([PIECE], dt.float32, name="bounce_in")
                nc.sync.dma_start(
                    bounce_in[QOFF:DHOFF].rearrange("(p b) -> p b", p=128),
                    q_piece[:])
                nc.sync.dma_start(
                    bounce_in[DHOFF:CTXOFF].rearrange("(p b) -> p b", p=HD),
                    dh_piece[:])
                for b in range(BL):
                    r, g = b // 4, b % 4
                    nc.sync.dma_start(
                        bounce_in[CTXOFF + 512 * b: CTXOFF + 512 * (b + 1)]
                        .rearrange("(o d) -> o d", o=1),
                        ctx_scat[32 * g + r:32 * g + r + 1,
                                 512 * r:512 * (r + 1)])
                bounce_out = dpool.tile([NCORES, PIECE], dt.float32,
                                        name="bounce_out")
                nc.gpsimd.collective_compute(
                    "AllGather", ALU.bypass, replica_groups=RG,
                    ins=[bounce_in[:].opt()], outs=[bounce_out[:].opt()])

                # ============ post-AG assembly ============
                qT_new = lpool.tile([128, 8, B], dt.float32, name="qT_as")
                nc.sync.dma_start(
                    qT_new[:],
                    bounce_out[:, QOFF:DHOFF].rearrange("c (p b) -> p c b", p=128))
                dhT_new = lpool.tile([128, 4, B], dt.float32, name="dhT_as")
                bo_pair = bounce_out[:].rearrange("(c h) x -> c h x", h=2)
                for h in range(2):
                    nc.sync.dma_start(
                        dhT_new[64 * h:64 * (h + 1), :, :],
                        bo_pair[:, h, DHOFF:CTXOFF]
                        .rearrange("c (p b) -> p c b", p=HD))
                ctx_bm = lpool.tile([B, T_ENC], dt.float32, name="ctx_bm")
                for cc in range(NCORES):
                    nc.sync.dma_start(
                        ctx_bm[cc * BL:(cc + 1) * BL, :],
                        bounce_out[cc, CTXOFF:PIECE].rearrange("(b d) -> b d", b=BL))
                ps_cT = ps_sm.tile([128, 4, B], dt.float32, name="ps_cT", tag="sm")
                for j in range(4):
                    nc.tensor.transpose(ps_cT[:, j, :],
                                        ctx_bm[:, 128 * j:128 * (j + 1)],
                                        ident[0:B, 0:B])
                ctxT_new = lpool.tile([128, 4, B], dt.float32, name="ctxT_as")
                nc.vector.tensor_copy(ctxT_new[:], ps_cT[:])

                # ============ dec matmul (t) ============
                g_dec = ps_dec.tile([B, 4 * HD + 2], dt.float32, name="g_dec")
                dec_pieces = ([qT_new[:, j, :] for j in range(8)]
                              + [ctxT_new[:, j, :] for j in range(4)]
                              + [dhT_new[:, j, :] for j in range(4)]
                              + [ones_sb[:]])
                for kt in range(DEC_KT):
                    nc.tensor.matmul(g_dec[:], dec_pieces[kt], wd[:, kt, :],
                                     start=(kt == 0), stop=(kt == DEC_KT - 1))

                # ============ proj (t-1) ============
                if t >= 1:
                    ps_pj = ps_sm.tile([B, PR], dt.float32, name="ps_pj", tag="sm")
                    pj_pieces = ([dhT_new[:, j, :] for j in range(4)]
                                 + [ctxT[:, j, :] for j in range(4)]
                                 + [ones_sb[:]])
                    for kt in range(PROJ_KT):
                        nc.tensor.matmul(ps_pj[:], pj_pieces[kt], wp[:, kt, :],
                                         start=(kt == 0), stop=(kt == PROJ_KT - 1))
                    out_sb = nlpool.tile([B, PR], dt.float32, name="out_sb")
                    nc.scalar.copy(out_sb[:], ps_pj[:])
                    nc.sync.dma_start(outs_d[t - 1], out_sb[:])

                ps_dec_prev = g_dec
                ctxT = ctxT_new
                qT = qT_new
                dhT = dhT_new

            # ============ epilogue: dec(S-1) nonlins + proj(S-1) ============
            th_d = lstm_nonlin(ps_dec_prev, HD, c_dec, "ed")
            dso = nlpool.tile([B, HD + 2], dt.float32, name="ed_so")
            nc.scalar.activation(dso[:], ps_dec_prev[:, 3 * HD:4 * HD + 2],
                                 AF.Sigmoid)
            dh_bm = nlpool.tile([B, HD], dt.float32, name="ed_dh")
            nc.vector.tensor_tensor(dh_bm[:], dso[:, 0:HD], th_d[:], ALU.mult)
            ps_dhT = ps_sm.tile([128, B], dt.float32, name="ed_psdhT", tag="sm")
            nc.tensor.transpose(ps_dhT[0:HD, :], dh_bm[:], ident[0:B, 0:B])
            dh_piece = lpool.tile([HD, B], dt.float32, name="ed_dhp")
            nc.scalar.copy(dh_piece[:], ps_dhT[0:HD, :])
            eb_in = dpool.tile([HD * B], dt.float32, name="eb_in")
            nc.sync.dma_start(eb_in[:].rearrange("(p b) -> p b", p=HD), dh_piece[:])
            eb_out = dpool.tile([NCORES, HD * B], dt.float32, name="eb_out")
            nc.gpsimd.collective_compute(
                "AllGather", ALU.bypass, replica_groups=RG,
                ins=[eb_in[:].opt()], outs=[eb_out[:].opt()])
            dhT_l = lpool.tile([128, 4, B], dt.float32, name="ed_dhT")
            ebo_pair = eb_out[:].rearrange("(c h) x -> c h x", h=2)
            for h in range(2):
                nc.sync.dma_start(
                    dhT_l[64 * h:64 * (h + 1), :, :],
                    ebo_pair[:, h, :].rearrange("c (p b) -> p c b", p=HD))
            ps_pj = ps_sm.tile([B, PR], dt.float32, name="ed_pj", tag="sm")
            pj_pieces = ([dhT_l[:, j, :] for j in range(4)]
                         + [ctxT[:, j, :] for j in range(4)] + [ones_sb[:]])
            for kt in range(PROJ_KT):
                nc.tensor.matmul(ps_pj[:], pj_pieces[kt], wp[:, kt, :],
                                 start=(kt == 0), stop=(kt == PROJ_KT - 1))
            out_sb = nlpool.tile([B, PR], dt.float32, name="ed_out")
            nc.scalar.copy(out_sb[:], ps_pj[:])
            nc.sync.dma_start(outs_d[S - 1], out_sb[:])

    nc.compile()
    return nc


_NC_CACHE = {}


def run(inputs, S=S_FULL):
    from concourse.bass_utils import run_bass_kernel_spmd
    if S not in _NC_CACHE:
        _NC_CACHE[S] = build(S)
    nc = _NC_CACHE[S]
    in_maps = prep_inputs(inputs)
    res = run_bass_kernel_spmd(nc, in_maps, list(range(NCORES)))
    outs = np.concatenate([res.results[c]["outs"] for c in range(NCORES)],
                          axis=-1)  # [S, B, 160]
    aligns = np.concatenate([res.results[c]["aligns"] for c in range(NCORES)],
                            axis=1)  # [S, B, 512]
    return outs, aligns


def kernel(**inputs):
    outs, aligns = run(inputs, S_FULL)
    outputs = outs.transpose(1, 0, 2).reshape(B, -1, MEM_DIM).transpose(0, 2, 1)
    alignments = np.ascontiguousarray(aligns.transpose(1, 0, 2))
    return np.ascontiguousarray(outputs.astype(np.float32)), alignments.astype(np.float32)
